# revision 23
# baseline (speedup 1.0000x reference)
"""Trainium2 Bass kernel for nn_ActorCritic loss_fn.

Strategy
--------
The reference computes a reverse discounted-return scan over time (T=8192),
normalizes the returns by masked global mean/std, and reduces to two scalar
losses. Both losses are polynomial in a fixed set of 10 masked global sums
involving the raw (unnormalized) returns R:

    N     = sum(m)          S1   = sum(m*R)       S2   = sum(m*R^2)
    SV    = sum(m*V)        SRV  = sum(m*R*V)     SV2  = sum(m*V^2)
    SLP   = sum(m*lp)       SLPR = sum(m*lp*R)    SLPV = sum(m*lp*V)
    SE    = sum(m*e)

so the device kernel only needs ONE streaming pass over the inputs: compute R
on the fly and accumulate the 10 sums. The final scalar math happens on host
in float64 during the gather step.

Sharding: batch dim (4096) split 8 ways -> 512 columns per core; each core
streams its (8192, 512) shard once (68 MiB) -> memory-bound roofline.

Per-core kernel: time is tiled into 64 chunks of 128 rows (partition dim).
The reverse scan inside a chunk is a matmul with a lower-triangular matrix of
gamma powers: R_chunk = Mscan @ r_chunk; the cross-chunk carry is a rank-1
accumulation (gamma^(P-p) outer R_next[0,:]) into the same PSUM bank. Masked
sums use scalar_tensor_tensor's fused accum_out (product + free-dim reduction
in one DVE instruction), written into per-(stat, chunk) columns of an SBUF
accumulator; the host reduces the rest.

The program is raw Bass (manual semaphores, standalone wait_ge instructions):
this container's walrus build allows only ONE sync wait per instruction, which
rules out the Tile layer's fused-wait style entirely.

Pipeline (all engines run the 64 chunks in reverse time order):
    SP : streams 5 input tiles per 512-row super-chunk, double-buffered
    PE : scan matmul + rank-1 carry matmul per chunk (PSUM, 2 banks)
    ACT: PSUM -> SBUF copy of R (triple-buffered R_sb)
    DVE: 10 fused masked-product+reduce ops per chunk
"""

import numpy as np
from contextlib import ExitStack

GAMMA = 0.99
ALPHA = 0.01
EPS = 1e-8

T = 8192
B = 4096
NCORES = 8
BL = B // NCORES        # 512 batch columns per core
P = 128                 # time rows per scan chunk (SBUF partition dim)
KPC = 4                 # chunks per DMA super-chunk (512 rows = 1 MiB/f32 tensor)
NSUPER = T // (P * KPC)  # 16
NCHUNK = T // P          # 64

STATS = ("N", "S1", "SV", "SLP", "SE", "S2", "SRV", "SLPR", "SLPV", "SV2")
NSTAT = len(STATS)

_cache = {}


def _build_program():
    import concourse.bass as bass
    import concourse.mybir as mybir

    dt = mybir.dt
    f32 = dt.float32
    mult = mybir.AluOpType.mult
    Copy = mybir.ActivationFunctionType.Copy

    nc = bass.Bass()
    r_d = nc.dram_tensor("rewards", [T, BL], f32, kind="ExternalInput")
    v_d = nc.dram_tensor("value_estimates", [T, BL], f32, kind="ExternalInput")
    l_d = nc.dram_tensor("log_probs", [T, BL], f32, kind="ExternalInput")
    e_d = nc.dram_tensor("entropies", [T, BL], f32, kind="ExternalInput")
    m_d = nc.dram_tensor("to_include", [T, BL], dt.uint8, kind="ExternalInput")
    out_d = nc.dram_tensor("stats", [P, NSTAT * NCHUNK], f32, kind="ExternalOutput")

    # Mscan[p, q] = gamma^(q-p) for q >= p; lhsT[q, p] = Mscan[p, q] (lower tri)
    qi = np.arange(P)
    scan_np = np.tril(GAMMA ** (qi[:, None] - qi[None, :])).astype(np.float32)
    scan_d = nc.inline_tensor(scan_np, "scanmat")
    # carry weights gamma^(P-p), replicated to 128 rows so the DMA spans all
    # partitions (keeps the +16-per-DMA semaphore accounting uniform)
    pow_np = np.tile((GAMMA ** (P - qi)).astype(np.float32), (P, 1))
    pow_d = nc.inline_tensor(pow_np, "powvec")

    with ExitStack() as ctx:
        def sb(name, shape, dtype):
            return ctx.enter_context(nc.sbuf_tensor(name, list(shape), dtype))

        scan_sb = sb("scan_sb", (P, P), f32)
        pow_sb = sb("pow_sb", (P, P), f32)
        r4 = [sb(f"r4_{i}", (P, KPC, BL), f32) for i in range(2)]
        v4 = [sb(f"v4_{i}", (P, KPC, BL), f32) for i in range(2)]
        l4 = [sb(f"l4_{i}", (P, KPC, BL), f32) for i in range(2)]
        e4 = [sb(f"e4_{i}", (P, KPC, BL), f32) for i in range(2)]
        m4 = [sb(f"m4_{i}", (P, KPC, BL), dt.uint8) for i in range(2)]
        R_sb = [sb(f"R_sb_{i}", (P, BL), f32) for i in range(3)]
        mf = sb("mf", (P, BL), f32)
        P1 = sb("P1", (P, BL), f32)
        W = sb("W", (P, BL), f32)
        Q = sb("Q", (P, BL), f32)
        scrs = [sb(f"scr_{i}", (P, BL), f32) for i in range(6)]
        acc = sb("acc", (P, NSTAT * NCHUNK), f32)
        R_ps = [ctx.enter_context(nc.psum_tensor(f"R_ps_{i}", [P, BL], f32))
                for i in range(2)]

        def acol(stat, c):
            col = STATS.index(stat) * NCHUNK + c
            return acc[:, col:col + 1]

        with nc.Block() as block, \
                nc.semaphore("const_sem") as const_sem, \
                nc.semaphore("dma_even") as dma_even, \
                nc.semaphore("dma_odd") as dma_odd, \
                nc.semaphore("pe_done") as pe_done, \
                nc.semaphore("act_copy") as act_copy, \
                nc.semaphore("dve_done") as dve_done, \
                nc.semaphore("dma_out") as dma_out:
            # Per-parity input-DMA sems: a waiter must wait the FULL count of
            # increments issued on a sem (partial counts race: the 16 per-DMA
            # increments of concurrent DMAs interleave). Super-chunk s's 5
            # DMAs all land on dma_par[s%2]; SP's slot pacing guarantees
            # super s-2 is not issued until s's readers are done, so a full
            # count on the parity sem exactly means "super s fully loaded".
            dma_par = (dma_even, dma_odd)

            def nsame(s):
                # number of super-chunks with parity s%2 issued up to s
                return (NSUPER - 1 - s) // 2 + 1

            @block.sync
            def _(sync):
                sync.dma_start(out=scan_sb[:], in_=scan_d[:]).then_inc(const_sem, 16)
                sync.dma_start(out=pow_sb[:], in_=pow_d[:]).then_inc(const_sem, 16)
                for s in reversed(range(NSUPER)):
                    if s <= NSUPER - 3:
                        # slot s%2 still holds super-chunk s+2: wait for its
                        # readers (PE matmuls, DVE products) to finish
                        done = NCHUNK - KPC * (s + 2)
                        sync.wait_ge(pe_done, done)
                        sync.wait_ge(dve_done, done)
                    sl = s % 2
                    rows = slice(s * P * KPC, (s + 1) * P * KPC)
                    for dst, src in ((r4[sl], r_d), (v4[sl], v_d), (l4[sl], l_d),
                                     (e4[sl], e_d), (m4[sl], m_d)):
                        sync.dma_start(
                            out=dst[:],
                            in_=src[rows, :].rearrange("(k p) n -> p k n", p=P),
                        ).then_inc(dma_par[sl], 16)
                sync.wait_ge(dve_done, NCHUNK)
                sync.dma_start(out=out_d[:], in_=acc[:]).then_inc(dma_out, 16)
                sync.wait_ge(dma_out, 16)

            @block.tensor
            def _(pe):
                pe.wait_ge(const_sem, 32)
                for c in reversed(range(NCHUNK)):
                    s, k = divmod(c, KPC)
                    if k == KPC - 1:
                        pe.wait_ge(dma_par[s % 2], 80 * nsame(s))
                    rv = r4[s % 2][:, k, :]
                    ps = R_ps[c % 2]
                    if c == NCHUNK - 1:
                        mm = pe.matmul(ps[:], lhsT=scan_sb[:], rhs=rv,
                                       start=True, stop=True)
                    else:
                        pe.matmul(ps[:], lhsT=scan_sb[:], rhs=rv,
                                  start=True, stop=False)
                        pe.wait_ge(act_copy, NCHUNK - 1 - c)  # R_sb[c+1] ready
                        mm = pe.matmul(ps[:], lhsT=pow_sb[0:1, :],
                                       rhs=R_sb[(c + 1) % 3][0:1, :],
                                       start=False, stop=True)
                    mm.then_inc(pe_done, 1)

            @block.scalar
            def _(act):
                for c in reversed(range(NCHUNK)):
                    act.wait_ge(pe_done, NCHUNK - c)
                    if c <= NCHUNK - 4:
                        # R_sb slot c%3 still read by chunk c+3's DVE products
                        act.wait_ge(dve_done, NCHUNK - 3 - c)
                    act.activation(R_sb[c % 3][:], R_ps[c % 2][:], Copy) \
                        .then_inc(act_copy, 1)

            @block.vector
            def _(dve):
                for c in reversed(range(NCHUNK)):
                    s, k = divmod(c, KPC)
                    if k == KPC - 1:
                        dve.wait_ge(dma_par[s % 2], 80 * nsame(s))
                    dve.wait_ge(act_copy, NCHUNK - c)  # R_sb[c] ready
                    sl = s % 2
                    R = R_sb[c % 3][:]
                    vv = v4[sl][:, k, :]
                    lv = l4[sl][:, k, :]
                    ev = e4[sl][:, k, :]
                    mv = m4[sl][:, k, :]

                    def stt(out, in0, in1, stat):
                        return dve.scalar_tensor_tensor(
                            out=out, in0=in0, scalar=1.0, in1=in1,
                            op0=mult, op1=mult, accum_out=acol(stat, c),
                        )

                    stt(mf[:], mv, mv, "N")          # u8*u8 -> f32 mask + count
                    dve.drain()                      # flush mf before readers
                    stt(W[:], mf[:], vv, "SV")
                    stt(Q[:], mf[:], lv, "SLP")
                    stt(scrs[0][:], mf[:], ev, "SE")
                    stt(P1[:], mf[:], R, "S1")
                    dve.drain()                      # flush W/Q/P1 before level 2
                    stt(scrs[1][:], P1[:], R, "S2")
                    stt(scrs[2][:], P1[:], vv, "SRV")
                    stt(scrs[3][:], Q[:], R, "SLPR")
                    stt(scrs[4][:], Q[:], vv, "SLPV")
                    stt(scrs[5][:], W[:], vv, "SV2").then_inc(dve_done, 1)

    return nc


def _get_program():
    if "nc" not in _cache:
        _cache["nc"] = _build_program()
    return _cache["nc"]


def _shard_inputs(inputs):
    r = np.ascontiguousarray(inputs["rewards"], dtype=np.float32)
    v = np.ascontiguousarray(inputs["value_estimates"], dtype=np.float32)
    lp = np.ascontiguousarray(inputs["log_probs"], dtype=np.float32)
    e = np.ascontiguousarray(inputs["entropies"], dtype=np.float32)
    m = inputs["to_include"].astype(np.uint8)
    in_maps = []
    for c in range(NCORES):
        sl = slice(c * BL, (c + 1) * BL)
        in_maps.append({
            "rewards": np.ascontiguousarray(r[:, sl]),
            "value_estimates": np.ascontiguousarray(v[:, sl]),
            "log_probs": np.ascontiguousarray(lp[:, sl]),
            "entropies": np.ascontiguousarray(e[:, sl]),
            "to_include": np.ascontiguousarray(m[:, sl]),
        })
    return in_maps


def _execute(in_maps, trace=False):
    from concourse.bass_utils import run_bass_kernel_spmd

    nc = _get_program()
    res = run_bass_kernel_spmd(nc, in_maps, list(range(NCORES)), trace=trace)
    return res


def _stats_from_results(results):
    """results: list of per-core out dicts -> dict of 10 float64 global sums."""
    tot = {name: 0.0 for name in STATS}
    for cm in results:
        st = cm["stats"].astype(np.float64)
        for i, name in enumerate(STATS):
            tot[name] += st[:, i * NCHUNK:(i + 1) * NCHUNK].sum()
    return tot


def _finalize(tot):
    N = tot["N"]; S1 = tot["S1"]; S2 = tot["S2"]
    SV = tot["SV"]; SRV = tot["SRV"]; SV2 = tot["SV2"]
    SLP = tot["SLP"]; SLPR = tot["SLPR"]; SLPV = tot["SLPV"]; SE = tot["SE"]
    mean = S1 / N
    q = S2 - 2.0 * mean * S1 + mean * mean * N   # sum(m*(R-mean)^2)
    var = q / (N - 1.0)
    s = np.sqrt(var) + EPS
    critic = q / (s * s) - 2.0 * (SRV - mean * SV) / s + SV2
    actor = -(SLPR - mean * SLP) / s + SLPV - ALPHA * SE
    return (np.float32(critic), np.float32(actor))


def kernel(**inputs):
    in_maps = _shard_inputs(inputs)
    res = _execute(in_maps, trace=False)
    tot = _stats_from_results(res.results)
    return _finalize(tot)


# revision 29
# speedup vs baseline: 1.2233x; 1.2233x over previous
"""Trainium2 Bass kernel for nn_ActorCritic loss_fn.

Strategy
--------
The reference computes a reverse discounted-return scan over time (T=8192),
normalizes the returns by masked global mean/std, and reduces to two scalar
losses. Both losses are polynomial in 10 masked global sums involving the raw
(unnormalized) returns R:

    N     = sum(m)          S1   = sum(m*R)       S2   = sum(m*R^2)
    SV    = sum(m*V)        SRV  = sum(m*R*V)     SV2  = sum(m*V^2)
    SLP   = sum(m*lp)       SLPR = sum(m*lp*R)    SLPV = sum(m*lp*V)
    SE    = sum(m*e)

so the device kernel is ONE streaming pass: compute R on the fly, form masked
products, reduce. Final scalar math happens on host in float64.

Sharding: batch dim split 8 ways -> (8192, 512) per core, streamed once.

Per-core pipeline (time tiled into 64 chunks of 128 rows = partition dim):
  SP  : streams inputs per 512-row super-chunk, double-buffered.
        rewards as f32(r); V/lp/e/mask pre-cast to bf16 on host.
  PE  : reverse scan per chunk as fp32r matmul with a lower-triangular
        gamma-power matrix; cross-chunk carry as a second fp32r matmul with a
        row-selector matrix (Sel[q,p] = gamma^(P-p) iff q==0) against the
        previous chunk's f32 R tile, accumulated into the same PSUM bank.
        Also: 8 of the 10 stat reductions as bf16 ones-column matmuls
        accumulating into a shared PSUM stats bank across all 64 chunks.
  ACT : copies R PSUM->SBUF twice (f32r for the carry chain, bf16 for the
        products) and does 2 stat reductions via activation accum_out.
  DVE : 7 bf16 elementwise products (2x perf mode).
  GPS : 2 bf16 elementwise products.

Raw Bass with manual semaphores and standalone wait_ge instructions: this
container's walrus build allows only ONE sync wait per instruction, which
rules out the Tile layer's fused-wait style.

Numerics: products in bf16 with f32 accumulation; scan in fp32r (~2e-5 rms).
Expected end-to-end relative error ~1e-5 vs the f32 reference.
"""

import numpy as np
from contextlib import ExitStack

GAMMA = 0.99
ALPHA = 0.01
EPS = 1e-8

T = 8192
B = 4096
NCORES = 8
BL = B // NCORES        # 512 batch columns per core
P = 128                 # time rows per scan chunk (SBUF partition dim)
KPC = 4                 # chunks per DMA super-chunk (512 rows)
NSUPER = T // (P * KPC)  # 16
NCHUNK = T // P          # 64

# PE-reduced stats (rows of the PSUM stats bank, via ones-column matmuls)
PE_STATS = ("N", "S1", "SV", "SLP", "S2", "SRV", "SLPR", "SV2")
NPE = len(PE_STATS)
# ACT-reduced stats (activation accum_out, per-chunk columns in `acc`)
ACT_STATS = ("SE", "SLPV")
NACT = len(ACT_STATS)

_cache = {}


def _build_program():
    import concourse.bass as bass
    import concourse.mybir as mybir

    dt = mybir.dt
    f32 = dt.float32
    f32r = dt.float32r
    bf16 = dt.bfloat16
    mult = mybir.AluOpType.mult
    Copy = mybir.ActivationFunctionType.Copy

    nc = bass.Bass()
    r_d = nc.dram_tensor("rewards", [T, BL], f32r, kind="ExternalInput")
    v_d = nc.dram_tensor("value_estimates", [T, BL], bf16, kind="ExternalInput")
    l_d = nc.dram_tensor("log_probs", [T, BL], bf16, kind="ExternalInput")
    e_d = nc.dram_tensor("entropies", [T, BL], bf16, kind="ExternalInput")
    m_d = nc.dram_tensor("to_include", [T, BL], bf16, kind="ExternalInput")
    acc_d = nc.dram_tensor("acc_out", [P, NACT * NCHUNK], f32, kind="ExternalOutput")
    pes_d = nc.dram_tensor("pe_stats", [NPE, BL], f32, kind="ExternalOutput")

    qi = np.arange(P)
    # scan lhsT[q, p] = gamma^(q-p) for q >= p (lower triangular)
    scan_np = np.tril(GAMMA ** (qi[:, None] - qi[None, :])).astype(np.float32)
    scan_d = nc.inline_tensor(scan_np, "scanmat")
    # carry selector lhsT[q, p] = gamma^(P-p) iff q == 0:
    # out[p, n] = gamma^(P-p) * R_next[0, n]
    sel_np = np.zeros((P, P), dtype=np.float32)
    sel_np[0, :] = GAMMA ** (P - qi)
    sel_d = nc.inline_tensor(sel_np, "selmat")
    # ones-column matrices for the PE stat reductions: oneh[:, j*NPE + k] = (k == j)
    import ml_dtypes
    oneh_np = np.zeros((P, NPE * NPE), dtype=np.float32)
    for j in range(NPE):
        oneh_np[:, j * NPE + j] = 1.0
    oneh_d = nc.inline_tensor(oneh_np.astype(ml_dtypes.bfloat16), "onehmat")

    with ExitStack() as ctx:
        def sb(name, shape, dtype):
            return ctx.enter_context(nc.sbuf_tensor(name, list(shape), dtype))

        scan_sb = sb("scan_sb", (P, P), f32r)
        sel_sb = sb("sel_sb", (P, P), f32r)
        oneh_sb = sb("oneh_sb", (P, NPE * NPE), bf16)
        r4 = [sb(f"r4_{i}", (P, KPC, BL), f32r) for i in range(2)]
        v4 = [sb(f"v4_{i}", (P, KPC, BL), bf16) for i in range(2)]
        l4 = [sb(f"l4_{i}", (P, KPC, BL), bf16) for i in range(2)]
        e4 = [sb(f"e4_{i}", (P, KPC, BL), bf16) for i in range(2)]
        m4 = [sb(f"m4_{i}", (P, KPC, BL), bf16) for i in range(2)]
        R_sb = [sb(f"R_sb_{i}", (P, BL), f32r) for i in range(3)]
        R_bf = [sb(f"R_bf_{i}", (P, BL), bf16) for i in range(2)]
        # DVE products (double-buffered by chunk parity)
        mR = [sb(f"mR_{i}", (P, BL), bf16) for i in range(2)]
        mV = [sb(f"mV_{i}", (P, BL), bf16) for i in range(2)]
        mL = [sb(f"mL_{i}", (P, BL), bf16) for i in range(2)]
        pRR = [sb(f"pRR_{i}", (P, BL), bf16) for i in range(2)]
        pRV = [sb(f"pRV_{i}", (P, BL), bf16) for i in range(2)]
        pLR = [sb(f"pLR_{i}", (P, BL), bf16) for i in range(2)]
        pLV = [sb(f"pLV_{i}", (P, BL), bf16) for i in range(2)]
        # GPS products
        pME = [sb(f"pME_{i}", (P, BL), bf16) for i in range(2)]
        pVV = [sb(f"pVV_{i}", (P, BL), bf16) for i in range(2)]
        acc = sb("acc", (P, NACT * NCHUNK), f32)
        stats_sb = sb("stats_sb", (NPE, BL), f32)
        R_ps = [ctx.enter_context(nc.psum_tensor(f"R_ps_{i}", [P, BL], f32))
                for i in range(2)]
        st_ps = ctx.enter_context(nc.psum_tensor("st_ps", [NPE, BL], f32))

        def acol(stat, c):
            col = ACT_STATS.index(stat) * NCHUNK + c
            return acc[:, col:col + 1]

        def nsame(s):
            return (NSUPER - 1 - s) // 2 + 1

        with nc.Block() as block, \
                nc.semaphore("const_sem") as const_sem, \
                nc.semaphore("dma_even") as dma_even, \
                nc.semaphore("dma_odd") as dma_odd, \
                nc.semaphore("pe_scan") as pe_scan, \
                nc.semaphore("pe_done") as pe_done, \
                nc.semaphore("act_rc") as act_rc, \
                nc.semaphore("act_red") as act_red, \
                nc.semaphore("dve_l1") as dve_l1, \
                nc.semaphore("dve_l2") as dve_l2, \
                nc.semaphore("gps_done") as gps_done, \
                nc.semaphore("act_fin") as act_fin, \
                nc.semaphore("dma_out") as dma_out:
            dma_par = (dma_even, dma_odd)

            @block.sync
            def _(sync):
                sync.dma_start(out=scan_sb[:], in_=scan_d[:].bitcast(f32r)).then_inc(const_sem, 16)
                sync.dma_start(out=sel_sb[:], in_=sel_d[:].bitcast(f32r)).then_inc(const_sem, 16)
                sync.dma_start(out=oneh_sb[:], in_=oneh_d[:]).then_inc(const_sem, 16)
                for s in reversed(range(NSUPER)):
                    if s <= NSUPER - 3:
                        done = NCHUNK - KPC * (s + 2)
                        sync.wait_ge(pe_done, done)      # PE reduce groups (r4, m4)
                        sync.wait_ge(dve_l1, done)       # DVE level-1 (v4, l4, m4)
                        sync.wait_ge(gps_done, 2 * done)  # GPS products (m4, e4)
                    sl = s % 2
                    rows = slice(s * P * KPC, (s + 1) * P * KPC)
                    for dst, src in ((r4[sl], r_d), (v4[sl], v_d), (l4[sl], l_d),
                                     (e4[sl], e_d), (m4[sl], m_d)):
                        sync.dma_start(
                            out=dst[:],
                            in_=src[rows, :].rearrange("(k p) n -> p k n", p=P),
                        ).then_inc(dma_par[sl], 16)
                sync.wait_ge(act_red, NACT * NCHUNK)
                sync.wait_ge(act_fin, 1)
                sync.dma_start(out=acc_d[:], in_=acc[:]).then_inc(dma_out, 16)
                sync.dma_start(out=pes_d[:], in_=stats_sb[:]).then_inc(dma_out, 16)
                sync.wait_ge(dma_out, 32)

            def pe_reduces(pe, c):
                """stat-reduction matmuls for chunk c (emitted one iter later)"""
                s, k = divmod(c, KPC)
                sl, par = s % 2, c % 2
                pe.wait_ge(dve_l2, NCHUNK - c)
                pe.wait_ge(gps_done, 2 * (NCHUNK - c))
                srcs = {
                    "N": m4[sl][:, k, :], "S1": mR[par][:], "SV": mV[par][:],
                    "SLP": mL[par][:], "S2": pRR[par][:], "SRV": pRV[par][:],
                    "SLPR": pLR[par][:], "SV2": pVV[par][:],
                }
                start = c == NCHUNK - 1
                for j, stat in enumerate(PE_STATS):
                    mm = pe.matmul(st_ps[:], lhsT=oneh_sb[:, j * NPE:(j + 1) * NPE],
                                   rhs=srcs[stat],
                                   start=(start and j == 0),
                                   stop=(c == 0 and j == NPE - 1))
                    if stat == PE_STATS[-1]:
                        mm.then_inc(pe_done, 1)

            @block.tensor
            def _(pe):
                pe.wait_ge(const_sem, 48)
                for c in reversed(range(NCHUNK)):
                    s, k = divmod(c, KPC)
                    if k == KPC - 1:
                        pe.wait_ge(dma_par[s % 2], 80 * nsame(s))
                    if c <= NCHUNK - 3:
                        # R_ps bank c%2 must be fully drained by ACT (conv of c+2)
                        pe.wait_ge(act_rc, 2 * (NCHUNK - 2 - c))
                    rv = r4[s % 2][:, k, :]
                    ps = R_ps[c % 2]
                    if c == NCHUNK - 1:
                        mm = pe.matmul(ps[:], lhsT=scan_sb[:], rhs=rv,
                                       start=True, stop=True)
                    else:
                        pe.matmul(ps[:], lhsT=scan_sb[:], rhs=rv,
                                  start=True, stop=False)
                        # R_sb[c+1] written by ACT copy (odd act_rc increments)
                        pe.wait_ge(act_rc, 2 * (NCHUNK - 2 - c) + 1)
                        mm = pe.matmul(ps[:], lhsT=sel_sb[:],
                                       rhs=R_sb[(c + 1) % 3][:],
                                       start=False, stop=True)
                    mm.then_inc(pe_scan, 1)
                    if c < NCHUNK - 1:
                        pe_reduces(pe, c + 1)
                pe_reduces(pe, 0)

            def act_reduces(act, c):
                # in-place copies: out == in avoids an unsynced scratch tile;
                # the accum_out is the real result
                par = c % 2
                act.wait_ge(gps_done, 2 * (NCHUNK - c) - 1)  # pME of chunk c
                act.activation(pME[par][:], pME[par][:], Copy,
                               accum_out=acol("SE", c)).then_inc(act_red, 1)
                act.wait_ge(dve_l2, NCHUNK - c)
                act.activation(pLV[par][:], pLV[par][:], Copy,
                               accum_out=acol("SLPV", c)).then_inc(act_red, 1)

            @block.scalar
            def _(act):
                for c in reversed(range(NCHUNK)):
                    act.wait_ge(pe_scan, NCHUNK - c)
                    # R_sb slot WAR: rank1 of c+2 read slot (c+3)%3 == c%3
                    # covered by pe_scan wait above (rank1(c) done => rank1(c+2) done)
                    act.activation(R_sb[c % 3][:], R_ps[c % 2][:], Copy) \
                        .then_inc(act_rc, 1)
                    if c <= NCHUNK - 3:
                        # R_bf slot WAR: DVE level-1 of chunk c+2 read R_bf[c%2]
                        act.wait_ge(dve_l1, NCHUNK - 2 - c)
                    act.activation(R_bf[c % 2][:], R_ps[c % 2][:], Copy) \
                        .then_inc(act_rc, 1)
                    if c < NCHUNK - 1:
                        act_reduces(act, c + 1)
                act_reduces(act, 0)
                act.wait_ge(pe_done, NCHUNK)
                act.activation(stats_sb[:], st_ps[:], Copy).then_inc(act_fin, 1)

            @block.vector
            def _(dve):
                for c in reversed(range(NCHUNK)):
                    s, k = divmod(c, KPC)
                    sl, par = s % 2, c % 2
                    if k == KPC - 1:
                        dve.wait_ge(dma_par[sl], 80 * nsame(s))
                    dve.wait_ge(act_rc, 2 * (NCHUNK - c))  # R_bf[c] ready
                    if c <= NCHUNK - 3:
                        # product tiles (par) reused from chunk c+2: readers
                        dve.wait_ge(pe_done, NCHUNK - 2 - c)
                        dve.wait_ge(act_red, NACT * (NCHUNK - 2 - c))
                        dve.wait_ge(gps_done, 2 * (NCHUNK - 2 - c))
                    mv_in = m4[sl][:, k, :]
                    dve.tensor_tensor(out=mR[par][:], in0=mv_in, in1=R_bf[par][:], op=mult)
                    dve.tensor_tensor(out=mV[par][:], in0=mv_in, in1=v4[sl][:, k, :], op=mult)
                    dve.tensor_tensor(out=mL[par][:], in0=mv_in, in1=l4[sl][:, k, :], op=mult) \
                        .then_inc(dve_l1, 1)
                    dve.drain()
                    dve.tensor_tensor(out=pRR[par][:], in0=mR[par][:], in1=mR[par][:], op=mult)
                    dve.tensor_tensor(out=pRV[par][:], in0=mR[par][:], in1=mV[par][:], op=mult)
                    dve.tensor_tensor(out=pLR[par][:], in0=mL[par][:], in1=mR[par][:], op=mult)
                    dve.tensor_tensor(out=pLV[par][:], in0=mL[par][:], in1=mV[par][:], op=mult) \
                        .then_inc(dve_l2, 1)

            @block.gpsimd
            def _(gps):
                for c in reversed(range(NCHUNK)):
                    s, k = divmod(c, KPC)
                    sl, par = s % 2, c % 2
                    if k == KPC - 1:
                        gps.wait_ge(dma_par[sl], 80 * nsame(s))
                    if c <= NCHUNK - 3:
                        # pME/pVV tiles reused from c+2: readers ACT (SE) and PE (SV2)
                        gps.wait_ge(act_red, NACT * (NCHUNK - 2 - c))
                        gps.wait_ge(pe_done, NCHUNK - 2 - c)
                    gps.tensor_tensor(out=pME[par][:], in0=m4[sl][:, k, :],
                                      in1=e4[sl][:, k, :], op=mult).then_inc(gps_done, 1)
                    gps.wait_ge(dve_l1, NCHUNK - c)
                    gps.tensor_tensor(out=pVV[par][:], in0=mV[par][:],
                                      in1=mV[par][:], op=mult).then_inc(gps_done, 1)

    return nc


def _get_program():
    if "nc" not in _cache:
        _cache["nc"] = _build_program()
    return _cache["nc"]


def _shard_inputs(inputs):
    import ml_dtypes

    bf16 = ml_dtypes.bfloat16
    r = np.ascontiguousarray(inputs["rewards"], dtype=np.float32)
    v = np.asarray(inputs["value_estimates"], dtype=np.float32).astype(bf16)
    lp = np.asarray(inputs["log_probs"], dtype=np.float32).astype(bf16)
    e = np.asarray(inputs["entropies"], dtype=np.float32).astype(bf16)
    m = inputs["to_include"].astype(bf16)
    in_maps = []
    for c in range(NCORES):
        sl = slice(c * BL, (c + 1) * BL)
        in_maps.append({
            "rewards": np.ascontiguousarray(r[:, sl]),
            "value_estimates": np.ascontiguousarray(v[:, sl]),
            "log_probs": np.ascontiguousarray(lp[:, sl]),
            "entropies": np.ascontiguousarray(e[:, sl]),
            "to_include": np.ascontiguousarray(m[:, sl]),
        })
    return in_maps


def _execute(in_maps, trace=False):
    from concourse.bass_utils import run_bass_kernel_spmd

    nc = _get_program()
    res = run_bass_kernel_spmd(nc, in_maps, list(range(NCORES)), trace=trace)
    return res


def _stats_from_results(results):
    tot = {name: 0.0 for name in PE_STATS + ACT_STATS}
    for cm in results:
        pes = cm["pe_stats"].astype(np.float64)
        for j, name in enumerate(PE_STATS):
            tot[name] += pes[j].sum()
        ac = cm["acc_out"].astype(np.float64)
        for i, name in enumerate(ACT_STATS):
            tot[name] += ac[:, i * NCHUNK:(i + 1) * NCHUNK].sum()
    return tot


def _finalize(tot):
    N = tot["N"]; S1 = tot["S1"]; S2 = tot["S2"]
    SV = tot["SV"]; SRV = tot["SRV"]; SV2 = tot["SV2"]
    SLP = tot["SLP"]; SLPR = tot["SLPR"]; SLPV = tot["SLPV"]; SE = tot["SE"]
    mean = S1 / N
    q = S2 - 2.0 * mean * S1 + mean * mean * N   # sum(m*(R-mean)^2)
    var = q / (N - 1.0)
    s = np.sqrt(var) + EPS
    critic = q / (s * s) - 2.0 * (SRV - mean * SV) / s + SV2
    actor = -(SLPR - mean * SLP) / s + SLPV - ALPHA * SE
    return (np.float32(critic), np.float32(actor))


def kernel(**inputs):
    in_maps = _shard_inputs(inputs)
    res = _execute(in_maps, trace=False)
    tot = _stats_from_results(res.results)
    return _finalize(tot)


# revision 30
# speedup vs baseline: 1.2299x; 1.0054x over previous
"""Trainium2 Bass kernel for nn_ActorCritic loss_fn.

Strategy
--------
The reference computes a reverse discounted-return scan over time (T=8192),
normalizes the returns by masked global mean/std, and reduces to two scalar
losses. Both losses are polynomial in 10 masked global sums involving the raw
(unnormalized) returns R:

    N     = sum(m)          S1   = sum(m*R)       S2   = sum(m*R^2)
    SV    = sum(m*V)        SRV  = sum(m*R*V)     SV2  = sum(m*V^2)
    SLP   = sum(m*lp)       SLPR = sum(m*lp*R)    SLPV = sum(m*lp*V)
    SE    = sum(m*e)

so the device kernel is ONE streaming pass: compute R on the fly, form masked
products, reduce. Final scalar math happens on host in float64.

Sharding: batch dim split 8 ways -> (8192, 512) per core, streamed once.

Per-core pipeline (time tiled into 64 chunks of 128 rows = partition dim):
  SP  : streams inputs per 512-row super-chunk, double-buffered.
        rewards as f32(r); V/lp/e/mask pre-cast to bf16 on host.
  PE  : reverse scan per chunk as fp32r matmul with a lower-triangular
        gamma-power matrix; cross-chunk carry as a second fp32r matmul with a
        row-selector matrix (Sel[q,p] = gamma^(P-p) iff q==0) against the
        previous chunk's f32 R tile, accumulated into the same PSUM bank.
        Also: 8 of the 10 stat reductions as bf16 ones-column matmuls
        accumulating into a shared PSUM stats bank across all 64 chunks.
  ACT : copies R PSUM->SBUF twice (f32r for the carry chain, bf16 for the
        products) and does 2 stat reductions via activation accum_out.
  DVE : 7 bf16 elementwise products (2x perf mode).
  GPS : 2 bf16 elementwise products.

Raw Bass with manual semaphores and standalone wait_ge instructions: this
container's walrus build allows only ONE sync wait per instruction, which
rules out the Tile layer's fused-wait style.

Numerics: products in bf16 with f32 accumulation; scan in fp32r (~2e-5 rms).
Expected end-to-end relative error ~1e-5 vs the f32 reference.
"""

import numpy as np
from contextlib import ExitStack

GAMMA = 0.99
ALPHA = 0.01
EPS = 1e-8

T = 8192
B = 4096
NCORES = 8
BL = B // NCORES        # 512 batch columns per core
P = 128                 # time rows per scan chunk (SBUF partition dim)
KPC = 4                 # chunks per DMA super-chunk (512 rows)
NSUPER = T // (P * KPC)  # 16
NCHUNK = T // P          # 64

# PE-reduced stats (rows of the PSUM stats bank, via ones-column matmuls)
PE_STATS = ("N", "S1", "SV", "SLP", "S2", "SRV", "SLPR", "SV2")
NPE = len(PE_STATS)
# ACT-reduced stats (activation accum_out, per-chunk columns in `acc`)
ACT_STATS = ("SE", "SLPV")
NACT = len(ACT_STATS)

_cache = {}


def _build_program():
    import concourse.bass as bass
    import concourse.mybir as mybir

    dt = mybir.dt
    f32 = dt.float32
    f32r = dt.float32r
    bf16 = dt.bfloat16
    mult = mybir.AluOpType.mult
    Copy = mybir.ActivationFunctionType.Copy

    nc = bass.Bass()
    r_d = nc.dram_tensor("rewards", [T, BL], f32r, kind="ExternalInput")
    v_d = nc.dram_tensor("value_estimates", [T, BL], bf16, kind="ExternalInput")
    l_d = nc.dram_tensor("log_probs", [T, BL], bf16, kind="ExternalInput")
    e_d = nc.dram_tensor("entropies", [T, BL], bf16, kind="ExternalInput")
    m_d = nc.dram_tensor("to_include", [T, BL], bf16, kind="ExternalInput")
    acc_d = nc.dram_tensor("acc_out", [P, NACT * NCHUNK], f32, kind="ExternalOutput")
    pes_d = nc.dram_tensor("pe_stats", [NPE, BL], f32, kind="ExternalOutput")

    qi = np.arange(P)
    # scan lhsT[q, p] = gamma^(q-p) for q >= p (lower triangular)
    scan_np = np.tril(GAMMA ** (qi[:, None] - qi[None, :])).astype(np.float32)
    scan_d = nc.inline_tensor(scan_np, "scanmat")
    # carry selector lhsT[q, p] = gamma^(P-p) iff q == 0:
    # out[p, n] = gamma^(P-p) * R_next[0, n]
    sel_np = np.zeros((P, P), dtype=np.float32)
    sel_np[0, :] = GAMMA ** (P - qi)
    sel_d = nc.inline_tensor(sel_np, "selmat")
    # ones-column matrices for the PE stat reductions: oneh[:, j*NPE + k] = (k == j)
    import ml_dtypes
    oneh_np = np.zeros((P, NPE * NPE), dtype=np.float32)
    for j in range(NPE):
        oneh_np[:, j * NPE + j] = 1.0
    oneh_d = nc.inline_tensor(oneh_np.astype(ml_dtypes.bfloat16), "onehmat")

    with ExitStack() as ctx:
        def sb(name, shape, dtype):
            return ctx.enter_context(nc.sbuf_tensor(name, list(shape), dtype))

        scan_sb = sb("scan_sb", (P, P), f32r)
        sel_sb = sb("sel_sb", (P, P), f32r)
        oneh_sb = sb("oneh_sb", (P, NPE * NPE), bf16)
        r4 = [sb(f"r4_{i}", (P, KPC, BL), f32r) for i in range(2)]
        v4 = [sb(f"v4_{i}", (P, KPC, BL), bf16) for i in range(2)]
        l4 = [sb(f"l4_{i}", (P, KPC, BL), bf16) for i in range(2)]
        e4 = [sb(f"e4_{i}", (P, KPC, BL), bf16) for i in range(2)]
        m4 = [sb(f"m4_{i}", (P, KPC, BL), bf16) for i in range(2)]
        R_sb = [sb(f"R_sb_{i}", (P, BL), f32r) for i in range(3)]
        R_bf = [sb(f"R_bf_{i}", (P, BL), bf16) for i in range(2)]
        # DVE products (double-buffered by chunk parity)
        mR = [sb(f"mR_{i}", (P, BL), bf16) for i in range(2)]
        mV = [sb(f"mV_{i}", (P, BL), bf16) for i in range(2)]
        mL = [sb(f"mL_{i}", (P, BL), bf16) for i in range(2)]
        pRR = [sb(f"pRR_{i}", (P, BL), bf16) for i in range(2)]
        pRV = [sb(f"pRV_{i}", (P, BL), bf16) for i in range(2)]
        pLR = [sb(f"pLR_{i}", (P, BL), bf16) for i in range(2)]
        pLV = [sb(f"pLV_{i}", (P, BL), bf16) for i in range(2)]
        # GPS products
        pME = [sb(f"pME_{i}", (P, BL), bf16) for i in range(2)]
        pVV = [sb(f"pVV_{i}", (P, BL), bf16) for i in range(2)]
        acc = sb("acc", (P, NACT * NCHUNK), f32)
        stats_sb = sb("stats_sb", (NPE, BL), f32)
        R_ps = [ctx.enter_context(nc.psum_tensor(f"R_ps_{i}", [P, BL], f32))
                for i in range(2)]
        st_ps = ctx.enter_context(nc.psum_tensor("st_ps", [NPE, BL], f32))

        def acol(stat, c):
            col = ACT_STATS.index(stat) * NCHUNK + c
            return acc[:, col:col + 1]

        def nsame(s):
            return (NSUPER - 1 - s) // 2 + 1

        with nc.Block() as block, \
                nc.semaphore("const_sem") as const_sem, \
                nc.semaphore("dma_even") as dma_even, \
                nc.semaphore("dma_odd") as dma_odd, \
                nc.semaphore("pe_scan") as pe_scan, \
                nc.semaphore("pe_done") as pe_done, \
                nc.semaphore("act_rc") as act_rc, \
                nc.semaphore("act_red") as act_red, \
                nc.semaphore("dve_l1") as dve_l1, \
                nc.semaphore("dve_l2") as dve_l2, \
                nc.semaphore("gps_done") as gps_done, \
                nc.semaphore("act_fin") as act_fin, \
                nc.semaphore("dma_out") as dma_out:
            dma_par = (dma_even, dma_odd)

            @block.sync
            def _(sync):
                sync.dma_start(out=scan_sb[:], in_=scan_d[:].bitcast(f32r)).then_inc(const_sem, 16)
                sync.dma_start(out=sel_sb[:], in_=sel_d[:].bitcast(f32r)).then_inc(const_sem, 16)
                sync.dma_start(out=oneh_sb[:], in_=oneh_d[:]).then_inc(const_sem, 16)
                for s in reversed(range(NSUPER)):
                    if s <= NSUPER - 3:
                        done = NCHUNK - KPC * (s + 2)
                        sync.wait_ge(pe_done, done)      # PE reduce groups (r4, m4)
                        sync.wait_ge(dve_l1, done)       # DVE level-1 (v4, l4, m4)
                        sync.wait_ge(gps_done, 2 * done)  # GPS products (m4, e4)
                    sl = s % 2
                    rows = slice(s * P * KPC, (s + 1) * P * KPC)
                    for dst, src in ((r4[sl], r_d), (v4[sl], v_d), (l4[sl], l_d),
                                     (e4[sl], e_d), (m4[sl], m_d)):
                        sync.dma_start(
                            out=dst[:],
                            in_=src[rows, :].rearrange("(k p) n -> p k n", p=P),
                        ).then_inc(dma_par[sl], 16)
                sync.wait_ge(act_red, NACT * NCHUNK)
                sync.wait_ge(act_fin, 1)
                sync.dma_start(out=acc_d[:], in_=acc[:]).then_inc(dma_out, 16)
                sync.dma_start(out=pes_d[:], in_=stats_sb[:]).then_inc(dma_out, 16)
                sync.wait_ge(dma_out, 32)

            def pe_reduces(pe, c):
                """stat-reduction matmuls for chunk c (emitted one iter later)"""
                s, k = divmod(c, KPC)
                sl, par = s % 2, c % 2
                pe.wait_ge(dve_l2, NCHUNK - c)
                pe.wait_ge(gps_done, 2 * (NCHUNK - c))
                srcs = {
                    "N": m4[sl][:, k, :], "S1": mR[par][:], "SV": mV[par][:],
                    "SLP": mL[par][:], "S2": pRR[par][:], "SRV": pRV[par][:],
                    "SLPR": pLR[par][:], "SV2": pVV[par][:],
                }
                start = c == NCHUNK - 1
                for j, stat in enumerate(PE_STATS):
                    mm = pe.matmul(st_ps[:], lhsT=oneh_sb[:, j * NPE:(j + 1) * NPE],
                                   rhs=srcs[stat],
                                   start=(start and j == 0),
                                   stop=(c == 0 and j == NPE - 1))
                    if stat == PE_STATS[-1]:
                        mm.then_inc(pe_done, 1)

            @block.tensor
            def _(pe):
                pe.wait_ge(const_sem, 48)
                for c in reversed(range(NCHUNK)):
                    s, k = divmod(c, KPC)
                    if k == KPC - 1:
                        pe.wait_ge(dma_par[s % 2], 80 * nsame(s))
                    if c <= NCHUNK - 3:
                        # R_ps bank c%2 must be fully drained by ACT (conv of c+2)
                        pe.wait_ge(act_rc, 2 * (NCHUNK - 2 - c))
                    rv = r4[s % 2][:, k, :]
                    ps = R_ps[c % 2]
                    if c == NCHUNK - 1:
                        mm = pe.matmul(ps[:], lhsT=scan_sb[:], rhs=rv,
                                       start=True, stop=True)
                    else:
                        pe.matmul(ps[:], lhsT=scan_sb[:], rhs=rv,
                                  start=True, stop=False)
                        # R_sb[c+1] written by ACT copy (odd act_rc increments)
                        pe.wait_ge(act_rc, 2 * (NCHUNK - 2 - c) + 1)
                        mm = pe.matmul(ps[:], lhsT=sel_sb[:],
                                       rhs=R_sb[(c + 1) % 3][:],
                                       start=False, stop=True)
                    mm.then_inc(pe_scan, 1)
                    if c < NCHUNK - 1:
                        pe_reduces(pe, c + 1)
                pe_reduces(pe, 0)

            def act_reduces(act, c):
                # in-place copies: out == in avoids an unsynced scratch tile;
                # the accum_out is the real result
                par = c % 2
                act.wait_ge(gps_done, 2 * (NCHUNK - c) - 1)  # pME of chunk c
                act.activation(pME[par][:], pME[par][:], Copy,
                               accum_out=acol("SE", c)).then_inc(act_red, 1)
                act.wait_ge(dve_l2, NCHUNK - c)
                act.activation(pLV[par][:], pLV[par][:], Copy,
                               accum_out=acol("SLPV", c)).then_inc(act_red, 1)

            @block.scalar
            def _(act):
                for c in reversed(range(NCHUNK)):
                    act.wait_ge(pe_scan, NCHUNK - c)
                    # R_sb slot WAR: rank1 of c+2 read slot (c+3)%3 == c%3
                    # covered by pe_scan wait above (rank1(c) done => rank1(c+2) done)
                    act.activation(R_sb[c % 3][:], R_ps[c % 2][:], Copy) \
                        .then_inc(act_rc, 1)
                    if c <= NCHUNK - 3:
                        # R_bf slot WAR: DVE level-1 of chunk c+2 read R_bf[c%2]
                        act.wait_ge(dve_l1, NCHUNK - 2 - c)
                    act.activation(R_bf[c % 2][:], R_ps[c % 2][:], Copy) \
                        .then_inc(act_rc, 1)
                    if c < NCHUNK - 1:
                        act_reduces(act, c + 1)
                act_reduces(act, 0)
                act.wait_ge(pe_done, NCHUNK)
                act.activation(stats_sb[:], st_ps[:], Copy).then_inc(act_fin, 1)

            @block.vector
            def _(dve):
                for c in reversed(range(NCHUNK)):
                    s, k = divmod(c, KPC)
                    sl, par = s % 2, c % 2
                    if k == KPC - 1:
                        dve.wait_ge(dma_par[sl], 80 * nsame(s))
                    dve.wait_ge(act_rc, 2 * (NCHUNK - c))  # R_bf[c] ready
                    if c <= NCHUNK - 3:
                        # product tiles (par) reused from chunk c+2: readers
                        dve.wait_ge(pe_done, NCHUNK - 2 - c)
                        dve.wait_ge(act_red, NACT * (NCHUNK - 2 - c))
                        dve.wait_ge(gps_done, 2 * (NCHUNK - 2 - c))
                    mv_in = m4[sl][:, k, :]
                    dve.tensor_tensor(out=mR[par][:], in0=mv_in, in1=R_bf[par][:], op=mult)
                    dve.tensor_tensor(out=mV[par][:], in0=mv_in, in1=v4[sl][:, k, :], op=mult)
                    dve.tensor_tensor(out=mL[par][:], in0=mv_in, in1=l4[sl][:, k, :], op=mult) \
                        .then_inc(dve_l1, 1)
                    # self-wait on dve_l1 orders level-2 after the level-1
                    # writes are committed (much cheaper than a full DRAIN)
                    dve.wait_ge(dve_l1, NCHUNK - c)
                    dve.tensor_tensor(out=pRR[par][:], in0=mR[par][:], in1=mR[par][:], op=mult)
                    dve.tensor_tensor(out=pRV[par][:], in0=mR[par][:], in1=mV[par][:], op=mult)
                    dve.tensor_tensor(out=pLR[par][:], in0=mL[par][:], in1=mR[par][:], op=mult)
                    dve.tensor_tensor(out=pLV[par][:], in0=mL[par][:], in1=mV[par][:], op=mult) \
                        .then_inc(dve_l2, 1)

            @block.gpsimd
            def _(gps):
                for c in reversed(range(NCHUNK)):
                    s, k = divmod(c, KPC)
                    sl, par = s % 2, c % 2
                    if k == KPC - 1:
                        gps.wait_ge(dma_par[sl], 80 * nsame(s))
                    if c <= NCHUNK - 3:
                        # pME/pVV tiles reused from c+2: readers ACT (SE) and PE (SV2)
                        gps.wait_ge(act_red, NACT * (NCHUNK - 2 - c))
                        gps.wait_ge(pe_done, NCHUNK - 2 - c)
                    gps.tensor_tensor(out=pME[par][:], in0=m4[sl][:, k, :],
                                      in1=e4[sl][:, k, :], op=mult).then_inc(gps_done, 1)
                    gps.wait_ge(dve_l1, NCHUNK - c)
                    gps.tensor_tensor(out=pVV[par][:], in0=mV[par][:],
                                      in1=mV[par][:], op=mult).then_inc(gps_done, 1)

    return nc


def _get_program():
    if "nc" not in _cache:
        _cache["nc"] = _build_program()
    return _cache["nc"]


def _shard_inputs(inputs):
    import ml_dtypes

    bf16 = ml_dtypes.bfloat16
    r = np.ascontiguousarray(inputs["rewards"], dtype=np.float32)
    v = np.asarray(inputs["value_estimates"], dtype=np.float32).astype(bf16)
    lp = np.asarray(inputs["log_probs"], dtype=np.float32).astype(bf16)
    e = np.asarray(inputs["entropies"], dtype=np.float32).astype(bf16)
    m = inputs["to_include"].astype(bf16)
    in_maps = []
    for c in range(NCORES):
        sl = slice(c * BL, (c + 1) * BL)
        in_maps.append({
            "rewards": np.ascontiguousarray(r[:, sl]),
            "value_estimates": np.ascontiguousarray(v[:, sl]),
            "log_probs": np.ascontiguousarray(lp[:, sl]),
            "entropies": np.ascontiguousarray(e[:, sl]),
            "to_include": np.ascontiguousarray(m[:, sl]),
        })
    return in_maps


def _execute(in_maps, trace=False):
    from concourse.bass_utils import run_bass_kernel_spmd

    nc = _get_program()
    res = run_bass_kernel_spmd(nc, in_maps, list(range(NCORES)), trace=trace)
    return res


def _stats_from_results(results):
    tot = {name: 0.0 for name in PE_STATS + ACT_STATS}
    for cm in results:
        pes = cm["pe_stats"].astype(np.float64)
        for j, name in enumerate(PE_STATS):
            tot[name] += pes[j].sum()
        ac = cm["acc_out"].astype(np.float64)
        for i, name in enumerate(ACT_STATS):
            tot[name] += ac[:, i * NCHUNK:(i + 1) * NCHUNK].sum()
    return tot


def _finalize(tot):
    N = tot["N"]; S1 = tot["S1"]; S2 = tot["S2"]
    SV = tot["SV"]; SRV = tot["SRV"]; SV2 = tot["SV2"]
    SLP = tot["SLP"]; SLPR = tot["SLPR"]; SLPV = tot["SLPV"]; SE = tot["SE"]
    mean = S1 / N
    q = S2 - 2.0 * mean * S1 + mean * mean * N   # sum(m*(R-mean)^2)
    var = q / (N - 1.0)
    s = np.sqrt(var) + EPS
    critic = q / (s * s) - 2.0 * (SRV - mean * SV) / s + SV2
    actor = -(SLPR - mean * SLP) / s + SLPV - ALPHA * SE
    return (np.float32(critic), np.float32(actor))


def kernel(**inputs):
    in_maps = _shard_inputs(inputs)
    res = _execute(in_maps, trace=False)
    tot = _stats_from_results(res.results)
    return _finalize(tot)


# revision 31
# speedup vs baseline: 1.2709x; 1.0334x over previous
"""Trainium2 Bass kernel for nn_ActorCritic loss_fn.

Strategy
--------
The reference computes a reverse discounted-return scan over time (T=8192),
normalizes the returns by masked global mean/std, and reduces to two scalar
losses. Both losses are polynomial in 10 masked global sums involving the raw
(unnormalized) returns R:

    N     = sum(m)          S1   = sum(m*R)       S2   = sum(m*R^2)
    SV    = sum(m*V)        SRV  = sum(m*R*V)     SV2  = sum(m*V^2)
    SLP   = sum(m*lp)       SLPR = sum(m*lp*R)    SLPV = sum(m*lp*V)
    SE    = sum(m*e)

so the device kernel is ONE streaming pass: compute R on the fly, form masked
products, reduce. Final scalar math happens on host in float64.

Sharding: batch dim split 8 ways -> (8192, 512) per core, streamed once.

Per-core pipeline (time tiled into 64 chunks of 128 rows = partition dim):
  SP  : streams inputs per 512-row super-chunk, double-buffered.
        rewards as f32(r); V/lp/e/mask pre-cast to bf16 on host.
  PE  : reverse scan per chunk as fp32r matmul with a lower-triangular
        gamma-power matrix; cross-chunk carry as a second fp32r matmul with a
        row-selector matrix (Sel[q,p] = gamma^(P-p) iff q==0) against the
        previous chunk's f32 R tile, accumulated into the same PSUM bank.
        Also: 8 of the 10 stat reductions as bf16 ones-column matmuls
        accumulating into a shared PSUM stats bank across all 64 chunks.
  ACT : copies R PSUM->SBUF twice (f32r for the carry chain, bf16 for the
        products) and does 2 stat reductions via activation accum_out.
  DVE : 7 bf16 elementwise products (2x perf mode).
  GPS : 2 bf16 elementwise products.

Raw Bass with manual semaphores and standalone wait_ge instructions: this
container's walrus build allows only ONE sync wait per instruction, which
rules out the Tile layer's fused-wait style.

Numerics: products in bf16 with f32 accumulation; scan in fp32r (~2e-5 rms).
Expected end-to-end relative error ~1e-5 vs the f32 reference.
"""

import numpy as np
from contextlib import ExitStack

GAMMA = 0.99
ALPHA = 0.01
EPS = 1e-8

T = 8192
B = 4096
NCORES = 8
BL = B // NCORES        # 512 batch columns per core
P = 128                 # time rows per scan chunk (SBUF partition dim)
KPC = 4                 # chunks per DMA super-chunk (512 rows)
NSUPER = T // (P * KPC)  # 16
NCHUNK = T // P          # 64

# PE-reduced stats (rows of the PSUM stats bank, via ones-column matmuls)
PE_STATS = ("N", "S1", "SV", "SLP", "S2", "SRV", "SLPR", "SV2")
NPE = len(PE_STATS)
# ACT-reduced stats (activation accum_out, per-chunk columns in `acc`)
ACT_STATS = ("SE", "SLPV")
NACT = len(ACT_STATS)

_cache = {}


def _build_program():
    import concourse.bass as bass
    import concourse.mybir as mybir

    dt = mybir.dt
    f32 = dt.float32
    f32r = dt.float32r
    bf16 = dt.bfloat16
    mult = mybir.AluOpType.mult
    Copy = mybir.ActivationFunctionType.Copy

    nc = bass.Bass()
    r_d = nc.dram_tensor("rewards", [T, BL], f32r, kind="ExternalInput")
    v_d = nc.dram_tensor("value_estimates", [T, BL], bf16, kind="ExternalInput")
    l_d = nc.dram_tensor("log_probs", [T, BL], bf16, kind="ExternalInput")
    e_d = nc.dram_tensor("entropies", [T, BL], bf16, kind="ExternalInput")
    m_d = nc.dram_tensor("to_include", [T, BL], bf16, kind="ExternalInput")
    acc_d = nc.dram_tensor("acc_out", [P, NACT * NCHUNK], f32, kind="ExternalOutput")
    pes_d = nc.dram_tensor("pe_stats", [NPE, BL], f32, kind="ExternalOutput")

    qi = np.arange(P)
    # scan lhsT[q, p] = gamma^(q-p) for q >= p (lower triangular)
    scan_np = np.tril(GAMMA ** (qi[:, None] - qi[None, :])).astype(np.float32)
    scan_d = nc.inline_tensor(scan_np, "scanmat")
    # carry selector lhsT[q, p] = gamma^(P-p) iff q == 0:
    # out[p, n] = gamma^(P-p) * R_next[0, n]
    sel_np = np.zeros((P, P), dtype=np.float32)
    sel_np[0, :] = GAMMA ** (P - qi)
    sel_d = nc.inline_tensor(sel_np, "selmat")
    # ones-column matrices for the PE stat reductions: oneh[:, j*NPE + k] = (k == j)
    import ml_dtypes
    oneh_np = np.zeros((P, NPE * NPE), dtype=np.float32)
    for j in range(NPE):
        oneh_np[:, j * NPE + j] = 1.0
    oneh_d = nc.inline_tensor(oneh_np.astype(ml_dtypes.bfloat16), "onehmat")

    with ExitStack() as ctx:
        def sb(name, shape, dtype):
            return ctx.enter_context(nc.sbuf_tensor(name, list(shape), dtype))

        scan_sb = sb("scan_sb", (P, P), f32r)
        sel_sb = sb("sel_sb", (P, P), f32r)
        oneh_sb = sb("oneh_sb", (P, NPE * NPE), bf16)
        r4 = [sb(f"r4_{i}", (P, KPC, BL), f32r) for i in range(2)]
        v4 = [sb(f"v4_{i}", (P, KPC, BL), bf16) for i in range(2)]
        l4 = [sb(f"l4_{i}", (P, KPC, BL), bf16) for i in range(2)]
        e4 = [sb(f"e4_{i}", (P, KPC, BL), bf16) for i in range(2)]
        m4 = [sb(f"m4_{i}", (P, KPC, BL), bf16) for i in range(2)]
        R_sb = [sb(f"R_sb_{i}", (P, BL), f32r) for i in range(3)]
        R_bf = [sb(f"R_bf_{i}", (P, BL), bf16) for i in range(2)]
        # DVE products (double-buffered by chunk parity)
        mR = [sb(f"mR_{i}", (P, BL), bf16) for i in range(2)]
        mV = [sb(f"mV_{i}", (P, BL), bf16) for i in range(2)]
        mL = [sb(f"mL_{i}", (P, BL), bf16) for i in range(2)]
        pRR = [sb(f"pRR_{i}", (P, BL), bf16) for i in range(2)]
        pRV = [sb(f"pRV_{i}", (P, BL), bf16) for i in range(2)]
        pLR = [sb(f"pLR_{i}", (P, BL), bf16) for i in range(2)]
        pLV = [sb(f"pLV_{i}", (P, BL), bf16) for i in range(2)]
        # GPS products
        pME = [sb(f"pME_{i}", (P, BL), bf16) for i in range(2)]
        pVV = [sb(f"pVV_{i}", (P, BL), bf16) for i in range(2)]
        acc = sb("acc", (P, NACT * NCHUNK), f32)
        stats_sb = sb("stats_sb", (NPE, BL), f32)
        R_ps = [ctx.enter_context(nc.psum_tensor(f"R_ps_{i}", [P, BL], f32))
                for i in range(2)]
        st_ps = ctx.enter_context(nc.psum_tensor("st_ps", [NPE, BL], f32))

        def acol(stat, c):
            col = ACT_STATS.index(stat) * NCHUNK + c
            return acc[:, col:col + 1]

        def nsame(s):
            return (NSUPER - 1 - s) // 2 + 1

        with nc.Block() as block, \
                nc.semaphore("const_sem") as const_sem, \
                nc.semaphore("dma_even") as dma_even, \
                nc.semaphore("dma_odd") as dma_odd, \
                nc.semaphore("pe_scan") as pe_scan, \
                nc.semaphore("pe_done") as pe_done, \
                nc.semaphore("act_rc") as act_rc, \
                nc.semaphore("act_red") as act_red, \
                nc.semaphore("dve_l1") as dve_l1, \
                nc.semaphore("dve_l2") as dve_l2, \
                nc.semaphore("gps_done") as gps_done, \
                nc.semaphore("act_fin") as act_fin, \
                nc.semaphore("dma_out") as dma_out:
            dma_par = (dma_even, dma_odd)

            @block.sync
            def _(sync):
                sync.dma_start(out=scan_sb[:], in_=scan_d[:].bitcast(f32r)).then_inc(const_sem, 16)
                sync.dma_start(out=sel_sb[:], in_=sel_d[:].bitcast(f32r)).then_inc(const_sem, 16)
                sync.dma_start(out=oneh_sb[:], in_=oneh_d[:]).then_inc(const_sem, 16)
                for s in reversed(range(NSUPER)):
                    if s <= NSUPER - 3:
                        done = NCHUNK - KPC * (s + 2)
                        sync.wait_ge(pe_done, done)      # PE reduce groups (r4, m4)
                        sync.wait_ge(dve_l1, done)       # DVE level-1 (v4, l4, m4)
                        sync.wait_ge(gps_done, 2 * done)  # GPS products (m4, e4)
                    sl = s % 2
                    rows = slice(s * P * KPC, (s + 1) * P * KPC)
                    for dst, src in ((r4[sl], r_d), (v4[sl], v_d), (l4[sl], l_d),
                                     (e4[sl], e_d), (m4[sl], m_d)):
                        sync.dma_start(
                            out=dst[:],
                            in_=src[rows, :].rearrange("(k p) n -> p k n", p=P),
                        ).then_inc(dma_par[sl], 16)
                sync.wait_ge(act_red, NACT * NCHUNK)
                sync.wait_ge(act_fin, 1)
                sync.dma_start(out=acc_d[:], in_=acc[:]).then_inc(dma_out, 16)
                sync.dma_start(out=pes_d[:], in_=stats_sb[:]).then_inc(dma_out, 16)
                sync.wait_ge(dma_out, 32)

            def pe_reduces(pe, c):
                """stat-reduction matmuls for chunk c (emitted one iter later)"""
                s, k = divmod(c, KPC)
                sl, par = s % 2, c % 2
                pe.wait_ge(dve_l2, NCHUNK - c)
                pe.wait_ge(gps_done, 2 * (NCHUNK - c))
                srcs = {
                    "N": m4[sl][:, k, :], "S1": mR[par][:], "SV": mV[par][:],
                    "SLP": mL[par][:], "S2": pRR[par][:], "SRV": pRV[par][:],
                    "SLPR": pLR[par][:], "SV2": pVV[par][:],
                }
                start = c == NCHUNK - 1
                for j, stat in enumerate(PE_STATS):
                    mm = pe.matmul(st_ps[:], lhsT=oneh_sb[:, j * NPE:(j + 1) * NPE],
                                   rhs=srcs[stat],
                                   start=(start and j == 0),
                                   stop=(c == 0 and j == NPE - 1))
                    if stat == PE_STATS[-1]:
                        mm.then_inc(pe_done, 1)

            @block.tensor
            def _(pe):
                pe.wait_ge(const_sem, 48)
                for c in reversed(range(NCHUNK)):
                    s, k = divmod(c, KPC)
                    if k == KPC - 1:
                        pe.wait_ge(dma_par[s % 2], 80 * nsame(s))
                    if c <= NCHUNK - 3:
                        # R_ps bank c%2 must be fully drained by ACT (conv of c+2)
                        pe.wait_ge(act_rc, 2 * (NCHUNK - 2 - c))
                    rv = r4[s % 2][:, k, :]
                    ps = R_ps[c % 2]
                    if c == NCHUNK - 1:
                        mm = pe.matmul(ps[:], lhsT=scan_sb[:], rhs=rv,
                                       start=True, stop=True)
                    else:
                        pe.matmul(ps[:], lhsT=scan_sb[:], rhs=rv,
                                  start=True, stop=False)
                        # R_sb[c+1] written by ACT copy (odd act_rc increments)
                        pe.wait_ge(act_rc, 2 * (NCHUNK - 2 - c) + 1)
                        mm = pe.matmul(ps[:], lhsT=sel_sb[:],
                                       rhs=R_sb[(c + 1) % 3][:],
                                       start=False, stop=True)
                    mm.then_inc(pe_scan, 1)
                    if c < NCHUNK - 2:
                        pe_reduces(pe, c + 2)
                pe_reduces(pe, 1)
                pe_reduces(pe, 0)

            def act_reduces(act, c):
                # in-place copies: out == in avoids an unsynced scratch tile;
                # the accum_out is the real result
                par = c % 2
                act.wait_ge(gps_done, 2 * (NCHUNK - c) - 1)  # pME of chunk c
                act.activation(pME[par][:], pME[par][:], Copy,
                               accum_out=acol("SE", c)).then_inc(act_red, 1)
                act.wait_ge(dve_l2, NCHUNK - c)
                act.activation(pLV[par][:], pLV[par][:], Copy,
                               accum_out=acol("SLPV", c)).then_inc(act_red, 1)

            @block.scalar
            def _(act):
                for c in reversed(range(NCHUNK)):
                    act.wait_ge(pe_scan, NCHUNK - c)
                    # R_sb slot WAR: rank1 of c+2 read slot (c+3)%3 == c%3
                    # covered by pe_scan wait above (rank1(c) done => rank1(c+2) done)
                    act.activation(R_sb[c % 3][:], R_ps[c % 2][:], Copy) \
                        .then_inc(act_rc, 1)
                    if c <= NCHUNK - 3:
                        # R_bf slot WAR: DVE level-1 of chunk c+2 read R_bf[c%2]
                        act.wait_ge(dve_l1, NCHUNK - 2 - c)
                    act.activation(R_bf[c % 2][:], R_ps[c % 2][:], Copy) \
                        .then_inc(act_rc, 1)
                    if c < NCHUNK - 2:
                        act_reduces(act, c + 2)
                act_reduces(act, 1)
                act_reduces(act, 0)
                act.wait_ge(pe_done, NCHUNK)
                act.activation(stats_sb[:], st_ps[:], Copy).then_inc(act_fin, 1)

            @block.vector
            def _(dve):
                for c in reversed(range(NCHUNK)):
                    s, k = divmod(c, KPC)
                    sl, par = s % 2, c % 2
                    if k == KPC - 1:
                        dve.wait_ge(dma_par[sl], 80 * nsame(s))
                    dve.wait_ge(act_rc, 2 * (NCHUNK - c))  # R_bf[c] ready
                    if c <= NCHUNK - 3:
                        # product tiles (par) reused from chunk c+2: readers
                        dve.wait_ge(pe_done, NCHUNK - 2 - c)
                        dve.wait_ge(act_red, NACT * (NCHUNK - 2 - c))
                        dve.wait_ge(gps_done, 2 * (NCHUNK - 2 - c))
                    mv_in = m4[sl][:, k, :]
                    dve.tensor_tensor(out=mR[par][:], in0=mv_in, in1=R_bf[par][:], op=mult)
                    dve.tensor_tensor(out=mV[par][:], in0=mv_in, in1=v4[sl][:, k, :], op=mult)
                    dve.tensor_tensor(out=mL[par][:], in0=mv_in, in1=l4[sl][:, k, :], op=mult) \
                        .then_inc(dve_l1, 1)
                    # self-wait on dve_l1 orders level-2 after the level-1
                    # writes are committed (much cheaper than a full DRAIN)
                    dve.wait_ge(dve_l1, NCHUNK - c)
                    dve.tensor_tensor(out=pRR[par][:], in0=mR[par][:], in1=mR[par][:], op=mult)
                    dve.tensor_tensor(out=pRV[par][:], in0=mR[par][:], in1=mV[par][:], op=mult)
                    dve.tensor_tensor(out=pLR[par][:], in0=mL[par][:], in1=mR[par][:], op=mult)
                    dve.tensor_tensor(out=pLV[par][:], in0=mL[par][:], in1=mV[par][:], op=mult) \
                        .then_inc(dve_l2, 1)

            @block.gpsimd
            def _(gps):
                for c in reversed(range(NCHUNK)):
                    s, k = divmod(c, KPC)
                    sl, par = s % 2, c % 2
                    if k == KPC - 1:
                        gps.wait_ge(dma_par[sl], 80 * nsame(s))
                    if c <= NCHUNK - 3:
                        # pME/pVV tiles reused from c+2: readers ACT (SE) and PE (SV2)
                        gps.wait_ge(act_red, NACT * (NCHUNK - 2 - c))
                        gps.wait_ge(pe_done, NCHUNK - 2 - c)
                    gps.tensor_tensor(out=pME[par][:], in0=m4[sl][:, k, :],
                                      in1=e4[sl][:, k, :], op=mult).then_inc(gps_done, 1)
                    gps.wait_ge(dve_l1, NCHUNK - c)
                    gps.tensor_tensor(out=pVV[par][:], in0=mV[par][:],
                                      in1=mV[par][:], op=mult).then_inc(gps_done, 1)

    return nc


def _get_program():
    if "nc" not in _cache:
        _cache["nc"] = _build_program()
    return _cache["nc"]


def _shard_inputs(inputs):
    import ml_dtypes

    bf16 = ml_dtypes.bfloat16
    r = np.ascontiguousarray(inputs["rewards"], dtype=np.float32)
    v = np.asarray(inputs["value_estimates"], dtype=np.float32).astype(bf16)
    lp = np.asarray(inputs["log_probs"], dtype=np.float32).astype(bf16)
    e = np.asarray(inputs["entropies"], dtype=np.float32).astype(bf16)
    m = inputs["to_include"].astype(bf16)
    in_maps = []
    for c in range(NCORES):
        sl = slice(c * BL, (c + 1) * BL)
        in_maps.append({
            "rewards": np.ascontiguousarray(r[:, sl]),
            "value_estimates": np.ascontiguousarray(v[:, sl]),
            "log_probs": np.ascontiguousarray(lp[:, sl]),
            "entropies": np.ascontiguousarray(e[:, sl]),
            "to_include": np.ascontiguousarray(m[:, sl]),
        })
    return in_maps


def _execute(in_maps, trace=False):
    from concourse.bass_utils import run_bass_kernel_spmd

    nc = _get_program()
    res = run_bass_kernel_spmd(nc, in_maps, list(range(NCORES)), trace=trace)
    return res


def _stats_from_results(results):
    tot = {name: 0.0 for name in PE_STATS + ACT_STATS}
    for cm in results:
        pes = cm["pe_stats"].astype(np.float64)
        for j, name in enumerate(PE_STATS):
            tot[name] += pes[j].sum()
        ac = cm["acc_out"].astype(np.float64)
        for i, name in enumerate(ACT_STATS):
            tot[name] += ac[:, i * NCHUNK:(i + 1) * NCHUNK].sum()
    return tot


def _finalize(tot):
    N = tot["N"]; S1 = tot["S1"]; S2 = tot["S2"]
    SV = tot["SV"]; SRV = tot["SRV"]; SV2 = tot["SV2"]
    SLP = tot["SLP"]; SLPR = tot["SLPR"]; SLPV = tot["SLPV"]; SE = tot["SE"]
    mean = S1 / N
    q = S2 - 2.0 * mean * S1 + mean * mean * N   # sum(m*(R-mean)^2)
    var = q / (N - 1.0)
    s = np.sqrt(var) + EPS
    critic = q / (s * s) - 2.0 * (SRV - mean * SV) / s + SV2
    actor = -(SLPR - mean * SLP) / s + SLPV - ALPHA * SE
    return (np.float32(critic), np.float32(actor))


def kernel(**inputs):
    in_maps = _shard_inputs(inputs)
    res = _execute(in_maps, trace=False)
    tot = _stats_from_results(res.results)
    return _finalize(tot)
